# revision 28
# baseline (speedup 1.0000x reference)
"""KL-attention kernel for Trainium2, 8-core data-parallel over batch.

Math (per batch b, x = [N=1024, D=1024] fp32):
  p = softmax(x, -1)
  S[i,j] = sum_d p[i,d] logp[j,d]  (attn = softmax(S, -1); row offsets cancel)
  Using sum_d p[i,d] = 1:  S[i,j] = (p @ x^T)[i,j] - logZ[j]
  out = softmax(S, -1) @ x

FP8 (e4m3) DoubleRow implementation (PE at 2x bf16 rate, 0.5 cyc/row):
  xb  = bf16(x)  casting DMA load (halves input traffic)
  xh  = fp8(x) (+ ones cols) casting DMA load; xl = fp8(xb - xh)   (DVE)
  eb  = fp8(exp(xb - ln4)) with f32 row-accumulate -> Z' = Z/4     (ACT)
  xt  = PE DoubleRow transpose of xh vs paired identity
  pt  = PE DoubleRow transpose of eb vs paired diag(1024/Z') ( = 1024*p^T )
  S^T = xt^T @ pt  via fp8 DR chains (PSUM f32)
  esf = bf16(exp(S/1024 + ln(256/Z')))   ( = 1024 * es, row bias on ACT )
  eh  = fp8(esf) casting DMA; el = fp8(esf - eh)                   (DVE)
  U   = eh@xh + eh@xl + el@xh  (error-compensated fp8 DR; el@xl ~0.1% dropped)
  z   = (eh + el) @ ones ; out = U * (1/z)                  (ACT copy*scale)

Per-operand fp8 quantization error either averages out (iid over the
contraction dim, ~0.2%) or is j-constant and cancels in the row softmax;
the hi/lo splits keep both matmuls at ~bf16 accuracy. Measured global
rel err ~5.8e-3 vs the fp32 reference (tolerance 2e-2).

Three-stage software pipeline: iteration b emits batch b's front end
(loads, exp, transposes, MM1, es split), batch b-2's MM2 (so the es hi/lo
split has a full iteration of slack), and batch b-1's z chains. The 16
transpose chains interleave with the 8 MM2 chains on the PE so PSUM-copy
latency is hidden; exp_eb(b+1) rides the ACT queue between outf ops.
ln(Z) for the exp bias is computed with one Newton step (linear seed +
Exp) so ACT only ever uses Exp/Copy and never reloads activation tables.
Engine split: ACT {exp_eb, exp_es, outf, 2 xt copies}, DVE {6 xt + 8 pt
PSUM copies, x_lo}, Pool {es_lo, diag build, SWDGE casting DMAs}.
"""

import os

import numpy as np

try:
    import concourse.bass as bass  # noqa: F401
except ImportError:
    import sys

    sys.path.insert(0, "/opt/trn_rl_repo")

from contextlib import ExitStack

import concourse.bass as bass
import concourse.mybir as mybir
import concourse.tile as tile
from concourse import bacc
from concourse.bass_utils import run_bass_kernel_spmd
from concourse.masks import make_identity

F32 = mybir.dt.float32
BF16 = mybir.dt.bfloat16
FP8 = mybir.dt.float8e4
AF = mybir.ActivationFunctionType
DR = mybir.MatmulPerfMode.DoubleRow

N_CORES = 8
B_PER_CORE = int(os.environ.get("KL_BPC", "4"))
N = 1024
D = 1024
P = 128
T = N // P  # 8 row tiles
TP = T // 2  # 4 tile pairs (DoubleRow contracts 2 k-tiles per instruction)
XB = D + 16  # data + ones cols (pair stride must be 16-byte aligned for DR)
LN4 = float(np.log(4.0))
ES_SCALE = 1024.0  # es stored as ES_SCALE*es so fp8(es) stays normal-range
C = 1024.0  # pt stored as C*p^T so fp8(diag) stays normal-range


def build_kernel_body(ctx: ExitStack, tc: "tile.TileContext", x_ap, out_ap):
    nc = tc.nc

    consts = ctx.enter_context(tc.tile_pool(name="consts", bufs=1))
    xbpool = ctx.enter_context(tc.tile_pool(name="xb", bufs=2))
    xhpool = ctx.enter_context(tc.tile_pool(name="xh", bufs=4))
    xlpool = ctx.enter_context(tc.tile_pool(name="xl", bufs=3))
    ebpool = ctx.enter_context(tc.tile_pool(name="eb", bufs=2))
    dgpool = ctx.enter_context(tc.tile_pool(name="dg", bufs=2))
    xtpool = ctx.enter_context(tc.tile_pool(name="xt", bufs=1))
    ptpool = ctx.enter_context(tc.tile_pool(name="pt", bufs=1))
    esfpool = ctx.enter_context(tc.tile_pool(name="esf", bufs=1))
    ehpool = ctx.enter_context(tc.tile_pool(name="eh", bufs=3))
    elpool = ctx.enter_context(tc.tile_pool(name="el", bufs=3))
    outpool = ctx.enter_context(tc.tile_pool(name="of", bufs=2))
    stats = ctx.enter_context(tc.tile_pool(name="st", bufs=2))
    mmpsum = ctx.enter_context(tc.tile_pool(name="mmps", bufs=4, space="PSUM"))

    ident_f = consts.tile([P, P], F32)
    make_identity(nc, ident_f[:, :])
    # identity scaled by C (for the pT diag blocks: dg = diag(C / Z'))
    ident_c = consts.tile([P, P], F32)
    nc.gpsimd.memset(ident_c[:, :], 0.0)
    nc.gpsimd.affine_select(
        out=ident_c[:, :],
        in_=ident_c[:, :],
        compare_op=mybir.AluOpType.not_equal,
        fill=C,
        base=0,
        pattern=[[-1, P]],
        channel_multiplier=1,
    )
    # paired identity for DoubleRow transposes: idp[:,0]=[I|0], idp[:,1]=[0|I]
    idp = consts.tile([P, 2, 2 * P], FP8)
    nc.gpsimd.memset(idp[:, :, :], 0.0)
    nc.gpsimd.tensor_copy(idp[:, 0, 0:P], ident_f[:, :])
    nc.gpsimd.tensor_copy(idp[:, 1, P : 2 * P], ident_f[:, :])
    # per-partition constant -ln4 (exp_eb bias; float imms need a const AP)
    nln4 = consts.tile([P, 1], F32)
    nc.gpsimd.memset(nln4[:, :], -LN4)

    H = T // 2
    B = B_PER_CORE
    states = []
    for b in range(B):
        states.append(
            dict(
                xh=xhpool.tile([P, T, XB], FP8, tag="xh", name=f"xh{b}"),
                xb=xbpool.tile([P, T, D], BF16, tag="xb", name=f"xb{b}"),
                xl=xlpool.tile([P, T, XB], FP8, tag="xl", name=f"xl{b}"),
                eb=ebpool.tile([P, T, D], FP8, tag="eb", name=f"eb{b}"),
                zs=stats.tile([P, T], F32, tag="zs", name=f"zs{b}"),
                rz=stats.tile([P, T], F32, tag="rz", name=f"rz{b}"),
                nlzb=stats.tile([P, T], F32, tag="nlzb", name=f"nlzb{b}"),
                dg=dgpool.tile([P, T, 2 * P], FP8, tag="dg", name=f"dg{b}"),
                xt=xtpool.tile([P, T, D], FP8, tag="xt", name=f"xt{b}"),
                pt=ptpool.tile([P, T, D], FP8, tag="pt", name=f"pt{b}"),
                esf=esfpool.tile([P, T, D], BF16, tag="esf", name=f"esf{b}"),
                eh=ehpool.tile([P, T, D], FP8, tag="eh", name=f"eh{b}"),
                el=elpool.tile([P, T, D], FP8, tag="el", name=f"el{b}"),
            )
        )

    def emit_loads(b, xb_first=False):
        # casting DMA loads (gpsimd SWDGE); the rearranged DRAM view puts
        # row r = t*128+p at [p, t, :]. For batch 0 the xb quarters go first
        # so exp_eb starts as soon as each tile lands.
        s = states[b]

        def load_xh():
            nc.gpsimd.dma_start(
                s["xh"][:, :, 0:D], x_ap[b].rearrange("(t p) d -> p t d", p=P)
            )

        def load_xb(nchunks):
            ch = T // nchunks
            for h in range(nchunks):
                nc.gpsimd.dma_start(
                    s["xb"][:, h * ch : (h + 1) * ch, :],
                    x_ap[b, h * ch * P : (h + 1) * ch * P, :].rearrange(
                        "(t p) d -> p t d", p=P
                    ),
                )

        if xb_first:
            load_xb(4)
            load_xh()
        else:
            load_xh()
            load_xb(2)
        nc.gpsimd.memset(s["xh"][:, :, D:XB], 1.0)

    def emit_exp_eb_one(b, t):
        s = states[b]
        nc.scalar.activation(
            s["eb"][:, t, :],
            s["xb"][:, t, :],
            AF.Exp,
            bias=nln4[:, 0:1],
            accum_out=s["zs"][:, t : t + 1],
        )

    def emit_rz_dg(b):
        """Row stats + diag blocks for batch b; ln via one Newton step so the
        ACT engine only ever uses Exp/Copy (no 1.3us table switches)."""
        s = states[b]
        nc.vector.reciprocal(s["rz"][:, :], s["zs"][:, :])
        # nlzb = ln(256*rz): y0n = -(linear approx); y1 = x*e^y0n - 1 - y0n
        y0n = stats.tile([P, T], F32, tag="y0n", name=f"y0n{b}")
        t1 = stats.tile([P, T], F32, tag="t1", name=f"t1{b}")
        tmp = stats.tile([P, T], F32, tag="tmp", name=f"tmp{b}")
        nc.vector.tensor_scalar(
            y0n[:, :],
            s["rz"][:, :],
            -256.0 * 1.62186,
            1.493791,
            mybir.AluOpType.mult,
            mybir.AluOpType.add,
        )
        nc.scalar.activation(t1[:, :], y0n[:, :], AF.Exp)
        nc.vector.scalar_tensor_tensor(
            tmp[:, :],
            s["rz"][:, :],
            256.0,
            t1[:, :],
            mybir.AluOpType.mult,
            mybir.AluOpType.mult,
        )
        nc.vector.scalar_tensor_tensor(
            s["nlzb"][:, :],
            tmp[:, :],
            -1.0,
            y0n[:, :],
            mybir.AluOpType.add,
            mybir.AluOpType.subtract,
        )
        # dg[:, t] = [diag(C*rz_t) | 0] even t, [0 | diag(C*rz_t)] odd t
        for t in range(T):
            h = t % 2
            nc.gpsimd.memset(s["dg"][:, t, (1 - h) * P : (2 - h) * P], 0.0)
            nc.gpsimd.tensor_scalar_mul(
                s["dg"][:, t, h * P : (h + 1) * P],
                ident_c[:, :],
                s["rz"][:, t : t + 1],
            )

    def emit_tx_one(b, k):
        s = states[b]
        ps_x = mmpsum.tile([P, D], F32, tag="ps")
        for m in range(TP):
            nc.tensor.matmul(
                ps_x[:, m * 2 * P : (m + 1) * 2 * P],
                s["xh"][:, 2 * m : 2 * m + 2, k * P : (k + 1) * P],
                idp[:, :, :],
                start=True,
                stop=True,
                perf_mode=DR,
            )
        if k < 2:
            nc.scalar.copy(s["xt"][:, k, :], ps_x[:, :])
        else:
            nc.vector.tensor_copy(s["xt"][:, k, :], ps_x[:, :])

    def emit_tp_one(b, k):
        s = states[b]
        ps_p = mmpsum.tile([P, D], F32, tag="ps")
        for m in range(TP):
            nc.tensor.matmul(
                ps_p[:, m * 2 * P : (m + 1) * 2 * P],
                s["eb"][:, 2 * m : 2 * m + 2, k * P : (k + 1) * P],
                s["dg"][:, 2 * m : 2 * m + 2, :],
                start=True,
                stop=True,
                perf_mode=DR,
            )
        nc.vector.tensor_copy(s["pt"][:, k, :], ps_p[:, :])

    def emit_mm1(b):
        s = states[b]
        for j in range(T):
            ps_s = mmpsum.tile([P, D], F32, tag="ps")
            for c in range(2):
                for m in range(TP):
                    nc.tensor.matmul(
                        ps_s[:, c * 512 : (c + 1) * 512],
                        s["xt"][:, 2 * m : 2 * m + 2, j * P : (j + 1) * P],
                        s["pt"][:, 2 * m : 2 * m + 2, c * 512 : (c + 1) * 512],
                        start=(m == 0),
                        stop=(m == TP - 1),
                        perf_mode=DR,
                    )
            nc.scalar.activation(
                s["esf"][:, j, :],
                ps_s[:, :],
                AF.Exp,
                bias=s["nlzb"][:, j : j + 1],
                scale=1.0 / C,
            )

    def emit_es_split(b):
        # eh via casting DMA; el on Pool so the DVE queue never carries
        # esf-gated work across the iteration boundary.
        s = states[b]
        Q = T // 4
        for q in range(4):
            nc.gpsimd.dma_start(  # casting DMA bf16 -> fp8
                s["eh"][:, q * Q : (q + 1) * Q, :],
                s["esf"][:, q * Q : (q + 1) * Q, :],
            )
            nc.gpsimd.tensor_sub(
                s["el"][:, q * Q : (q + 1) * Q, :],
                s["esf"][:, q * Q : (q + 1) * Q, :],
                s["eh"][:, q * Q : (q + 1) * Q, :],
            )

    def emit_xl(b):
        s = states[b]
        for h in range(2):
            nc.vector.tensor_sub(
                s["xl"][:, h * H : (h + 1) * H, 0:D],
                s["xb"][:, h * H : (h + 1) * H, :],
                s["xh"][:, h * H : (h + 1) * H, 0:D],
            )

    def emit_back_z(b):
        """z chains for batch b into one PSUM tile + reciprocal vector."""
        s = states[b]
        eh, el, xh = s["eh"], s["el"], s["xh"]
        ps_z = mmpsum.tile([P, P], F32, tag="ps")
        for i in range(T):
            step = 0
            for eA in (eh, el):
                for m in range(TP):
                    nc.tensor.matmul(
                        ps_z[:, i * 16 : (i + 1) * 16],
                        eA[:, 2 * m : 2 * m + 2, i * P : (i + 1) * P],
                        xh[:, 2 * m : 2 * m + 2, D:XB],
                        start=(step == 0),
                        stop=(step == 2 * TP - 1),
                        perf_mode=DR,
                    )
                    step += 1
        ziv = stats.tile([P, T], F32, tag="ziv")
        rziv = stats.tile([P, T], F32, tag="rziv")
        nc.vector.tensor_copy(ziv[:, :], ps_z[:, 0 : P : 16])
        nc.vector.reciprocal(rziv[:, :], ziv[:, :])
        s["rziv"] = rziv

    def emit_back_u_one(b, i):
        """One MM2 output tile (3-combo fp8), normalize, store."""
        s = states[b]
        eh, el, xh, xl, rziv = s["eh"], s["el"], s["xh"], s["xl"], s["rziv"]
        ps_o = mmpsum.tile([P, D], F32, tag="ps")
        for c in range(2):
            combos = [(eh, xh), (eh, xl), (el, xh)]
            last = len(combos) * TP - 1
            step = 0
            for eA, xA in combos:
                for m in range(TP):
                    nc.tensor.matmul(
                        ps_o[:, c * 512 : (c + 1) * 512],
                        eA[:, 2 * m : 2 * m + 2, i * P : (i + 1) * P],
                        xA[:, 2 * m : 2 * m + 2, c * 512 : (c + 1) * 512],
                        start=(step == 0),
                        stop=(step == last),
                        perf_mode=DR,
                    )
                    step += 1
        outf = outpool.tile([P, D], F32, tag="of")
        nc.scalar.activation(
            outf[:, :], ps_o[:, :], AF.Copy, scale=rziv[:, i : i + 1]
        )
        nc.sync.dma_start(out_ap[b, i * P : (i + 1) * P, :], outf[:, :])

    # Three-stage software pipeline: iteration b runs batch b's front end,
    # batch b-2's MM2 (so the es hi/lo split has a full iteration of slack),
    # and batch b-1's z chains. Transpose chains for b interleave with the
    # U chains of b-2 to cover PSUM-copy latency; exp_eb(b+1) rides the ACT
    # queue between outf ops.
    emit_loads(0, xb_first=True)
    for t in range(T):
        emit_exp_eb_one(0, t)
    emit_rz_dg(0)
    for b in range(B):
        if b + 1 < B:
            emit_loads(b + 1)
        for i in range(T):
            if b >= 2:
                emit_back_u_one(b - 2, i)
            if b + 1 < B:
                emit_exp_eb_one(b + 1, i)
            if i < 4:
                emit_tx_one(b, 2 * i)
                emit_tx_one(b, 2 * i + 1)
            else:
                emit_tp_one(b, 2 * (i - 4))
                emit_tp_one(b, 2 * (i - 4) + 1)
        if b >= 1:
            emit_back_z(b - 1)
        emit_mm1(b)
        if b + 1 < B:
            emit_rz_dg(b + 1)
        emit_es_split(b)
        emit_xl(b)
    for i in range(T):
        emit_back_u_one(B - 2, i)
    emit_back_z(B - 1)
    for i in range(T):
        emit_back_u_one(B - 1, i)


_CACHED = {}


def _build():
    if "nc" in _CACHED:
        return _CACHED["nc"]
    nc = bacc.Bacc(
        "TRN2",
        target_bir_lowering=False,
        debug=False,
        enable_asserts=False,
        num_devices=N_CORES,
    )
    x_ap = nc.dram_tensor("x", [B_PER_CORE, N, D], F32, kind="ExternalInput").ap()
    out_ap = nc.dram_tensor(
        "out", [B_PER_CORE, N, D], F32, kind="ExternalOutput"
    ).ap()
    with tile.TileContext(nc) as tc:
        with ExitStack() as ctx:
            build_kernel_body(ctx, tc, x_ap, out_ap)
    nc.compile()
    _CACHED["nc"] = nc
    return nc


LAST_EXEC_NS = None


def kernel(x: np.ndarray) -> np.ndarray:
    global LAST_EXEC_NS
    x = np.ascontiguousarray(np.asarray(x, dtype=np.float32))
    B = x.shape[0]
    assert B == N_CORES * B_PER_CORE and x.shape[1:] == (N, D)
    nc = _build()
    shards = x.reshape(N_CORES, B_PER_CORE, N, D)
    in_maps = [{"x": np.ascontiguousarray(shards[i])} for i in range(N_CORES)]
    trace = os.environ.get("KL_TRACE", "0") == "1"
    res = run_bass_kernel_spmd(
        nc, in_maps, core_ids=list(range(N_CORES)), trace=trace
    )
    LAST_EXEC_NS = res.exec_time_ns
    out = np.concatenate([r["out"] for r in res.results], axis=0)
    return out.astype(np.float32, copy=False)
